# revision 4
# baseline (speedup 1.0000x reference)
"""Bass/Trainium2 kernel for DegreeOnlyFiltration (segment max + gather-divide).

Contract: kernel(**inputs) takes FULL inputs (node_deg [N] f32, sample_pos
[G+1] i32 CSR boundaries) and returns the FULL output node_deg / seg_max.

Strategy (per the sharding hint): segments are contiguous; the expected input
has uniform boundaries (sample_pos = arange(G+1) * W).  We shard node_deg by
whole segments across the 8 NeuronCores (pure data parallel, no cross-core
traffic).  On each core the shard is [512, 4096] f32 (8 MiB); it is processed
as 4 chunks of [128 segments x 4096] (2 MiB, one segment per partition row,
16 KB DMA descriptors).  Engine roles are fixed so nothing blocks a DMA
doorbell:
  Sync (SP HWDGE ring):   all 4 input dma_starts, issued up front.
  DVE:                    reduce_max + reciprocal per chunk.
  ACT (ACT HWDGE ring):   in-place per-partition-scalar multiply per chunk,
                          immediately followed by that chunk's output
                          dma_start (in-order engine -> doorbell fires the
                          moment the mul retires).
The two HWDGE rings round-robin at the 16 SDMA engines so reads and writes
interleave; 16 KB descriptors (vs 8 KB) halve the descriptor-ring fetch load
on SDMA engine 15, which serves both rings' descriptor fetches and otherwise
straggles ~3 us past the other engines.
"""

import os

import numpy as np

import concourse.bacc as bacc
import concourse.mybir as mybir
import concourse.tile as tile
from concourse.bass_utils import run_bass_kernel_spmd

N_CORES = 8
P = 128  # SBUF partitions

# Populated after each traced run (test harness reads these).
LAST_EXEC_TIME_NS = None
LAST_RESULTS = None

_NC_CACHE = {}


def _build_uniform_nc(segs_per_core: int, width: int):
    """SPMD program: x [segs_per_core, width] f32 -> y = x / rowmax(x)."""
    assert segs_per_core % P == 0
    n_chunks = segs_per_core // P
    f32 = mybir.dt.float32

    nc = bacc.Bacc("TRN2", target_bir_lowering=False, debug=False,
                   num_devices=N_CORES, enable_partition_id=False,
                   enable_asserts=False)
    x = nc.dram_tensor("x", [segs_per_core, width], f32, kind="ExternalInput")
    y = nc.dram_tensor("y", [segs_per_core, width], f32, kind="ExternalOutput")

    # The first and last chunks' loads are split in two column-halves: the
    # first so its partial reduce starts ~3 us earlier (pipeline shifts
    # left), the last so its reduce starts before the final input packets
    # land.  Middle chunks load as single 2 MiB DMAs (16 KB descriptors).
    half = width // 2
    # Multiply is split DVE (3/8) / ACT (5/8): the reduce already occupies
    # DVE, and the two halves run concurrently so the chain is short.
    wd = (3 * width // 8) if width % 8 == 0 else 0

    with tile.TileContext(nc) as tc:
        with tc.tile_pool(name="p", bufs=1) as pool:
            # All input DMAs up front on the SP HWDGE ring (Sync engine does
            # nothing else, so every doorbell rings back-to-back).
            tins = []
            for t in range(n_chunks):
                split = (t in (0, n_chunks - 1)) and width % 2 == 0 \
                    and width >= 1024
                tin = pool.tile([P, width], f32, tag=f"tin{t}")
                src = x[t * P:(t + 1) * P, :]
                if split:
                    nc.sync.dma_start(tin[:, :half], src[:, :half])
                    nc.sync.dma_start(tin[:, half:], src[:, half:])
                else:
                    nc.sync.dma_start(tin[:], src)
                tins.append((tin, split))

            # DVE: rowmax + reciprocal per chunk (partial maxes for split
            # chunks), plus its share of the multiplies.  ACT: only its
            # share of the multiplies.  GpSimd: only the output doorbells
            # (SWDGE) -- an engine with nothing else scheduled on it rings
            # each doorbell the moment the muls' semaphores fire, and the
            # SWDGE descriptor path keeps the output descriptor fetches off
            # the HWDGE rings' descriptor-fetch engine (SDMA 15).
            for t in range(n_chunks):
                tin, split = tins[t]
                m = pool.tile([P, 1], f32, tag=f"m{t}")
                if split:
                    m2 = pool.tile([P, 2], f32, tag=f"m{t}p")
                    nc.vector.reduce_max(m2[:, 0:1], tin[:, :half],
                                         axis=mybir.AxisListType.X)
                    nc.vector.reduce_max(m2[:, 1:2], tin[:, half:],
                                         axis=mybir.AxisListType.X)
                    nc.vector.reduce_max(m[:], m2[:],
                                         axis=mybir.AxisListType.X)
                else:
                    nc.vector.reduce_max(m[:], tin[:],
                                         axis=mybir.AxisListType.X)
                r = pool.tile([P, 1], f32, tag=f"r{t}")
                nc.vector.reciprocal(r[:], m[:])
                if wd:
                    nc.vector.tensor_scalar_mul(tin[:, :wd], tin[:, :wd],
                                                r[:])
                    nc.scalar.mul(tin[:, wd:], tin[:, wd:], r[:])
                else:
                    nc.scalar.mul(tin[:], tin[:], r[:])
                nc.gpsimd.dma_start(y[t * P:(t + 1) * P, :], tin[:])
    nc.compile()
    return nc


def _uniform_width(sample_pos: np.ndarray, n: int):
    """Return segment width W if boundaries are uniform (pos = arange*W)."""
    if sample_pos[0] != 0 or sample_pos[-1] != n:
        return None
    diffs = np.diff(sample_pos)
    if diffs.size == 0 or np.any(diffs != diffs[0]):
        return None
    return int(diffs[0])


def _host_fallback(node_deg: np.ndarray, sample_pos: np.ndarray) -> np.ndarray:
    """Exact mirror of the reference semantics for non-uniform boundaries."""
    import jax

    with jax.default_device(jax.devices("cpu")[0]):
        import jax.numpy as jnp

        deg = jnp.asarray(node_deg)
        pos = jnp.asarray(sample_pos)
        n = deg.shape[0]
        g = pos.shape[0] - 1
        seg_ids = jnp.searchsorted(pos[1:], jnp.arange(n, dtype=pos.dtype),
                                   side="right")
        seg_max = jax.ops.segment_max(deg, seg_ids, num_segments=g)
        return np.asarray(deg / seg_max[seg_ids])


def kernel(node_deg: np.ndarray, sample_pos: np.ndarray) -> np.ndarray:
    global LAST_EXEC_TIME_NS, LAST_RESULTS

    node_deg = np.asarray(node_deg, dtype=np.float32)
    sample_pos = np.asarray(sample_pos, dtype=np.int32)
    n = node_deg.shape[0]
    g = sample_pos.shape[0] - 1

    width = _uniform_width(sample_pos, n)
    if (width is None or width < 512 or g % N_CORES != 0
            or (g // N_CORES) % P != 0):
        return _host_fallback(node_deg, sample_pos)

    segs_per_core = g // N_CORES

    key = (segs_per_core, width)
    if key not in _NC_CACHE:
        _NC_CACHE[key] = _build_uniform_nc(*key)
    nc = _NC_CACHE[key]

    shards = node_deg.reshape(N_CORES, segs_per_core, width)
    in_maps = [{"x": shards[c]} for c in range(N_CORES)]

    trace = bool(int(os.environ.get("KERNEL_TRACE", "0")))
    try:
        res = run_bass_kernel_spmd(nc, in_maps, core_ids=list(range(N_CORES)),
                                   trace=trace)
    except Exception:
        if not trace:
            raise
        # Trace post-processing can fail in sandboxes; results still matter.
        res = run_bass_kernel_spmd(nc, in_maps, core_ids=list(range(N_CORES)),
                                   trace=False)
    LAST_EXEC_TIME_NS = res.exec_time_ns
    LAST_RESULTS = res
    out = np.concatenate([res.results[c]["y"].reshape(-1)
                          for c in range(N_CORES)])
    return out.astype(np.float32, copy=False)


# revision 5
# speedup vs baseline: 1.1308x; 1.1308x over previous
"""Bass/Trainium2 kernel for DegreeOnlyFiltration (segment max + gather-divide).

Contract: kernel(**inputs) takes FULL inputs (node_deg [N] f32, sample_pos
[G+1] i32 CSR boundaries) and returns the FULL output node_deg / seg_max.

Strategy (per the sharding hint): segments are contiguous; the expected input
has uniform boundaries (sample_pos = arange(G+1) * W).  We shard node_deg by
whole segments across the 8 NeuronCores (pure data parallel, no cross-core
traffic).  On each core the shard is [512, 4096] f32 (8 MiB); it is processed
as 4 chunks of [128 segments x 4096] (2 MiB, one segment per partition row,
16 KB DMA descriptors).  Engine roles are fixed so nothing blocks a DMA
doorbell:
  Sync (SP HWDGE ring):   all 4 input dma_starts, issued up front.
  DVE:                    reduce_max + reciprocal per chunk.
  ACT (ACT HWDGE ring):   in-place per-partition-scalar multiply per chunk,
                          immediately followed by that chunk's output
                          dma_start (in-order engine -> doorbell fires the
                          moment the mul retires).
The two HWDGE rings round-robin at the 16 SDMA engines so reads and writes
interleave; 16 KB descriptors (vs 8 KB) halve the descriptor-ring fetch load
on SDMA engine 15, which serves both rings' descriptor fetches and otherwise
straggles ~3 us past the other engines.
"""

import os

import numpy as np

import concourse.bacc as bacc
import concourse.mybir as mybir
import concourse.tile as tile
from concourse.bass_utils import run_bass_kernel_spmd

N_CORES = 8
P = 128  # SBUF partitions

# Populated after each traced run (test harness reads these).
LAST_EXEC_TIME_NS = None
LAST_RESULTS = None

_NC_CACHE = {}


def _build_uniform_nc(segs_per_core: int, width: int):
    """SPMD program: x [segs_per_core, width] f32 -> y = x / rowmax(x)."""
    assert segs_per_core % P == 0
    n_chunks = segs_per_core // P
    f32 = mybir.dt.float32

    nc = bacc.Bacc("TRN2", target_bir_lowering=False, debug=False,
                   num_devices=N_CORES, enable_partition_id=False,
                   enable_asserts=False)
    x = nc.dram_tensor("x", [segs_per_core, width], f32, kind="ExternalInput")
    y = nc.dram_tensor("y", [segs_per_core, width], f32, kind="ExternalOutput")

    # The first and last chunks' loads are split in two column-halves: the
    # first so its partial reduce starts ~3 us earlier (pipeline shifts
    # left), the last so its reduce starts before the final input packets
    # land.  Middle chunks load as single 2 MiB DMAs (16 KB descriptors).
    half = width // 2
    # Multiply is split DVE (3/8) / ACT (5/8): the reduce already occupies
    # DVE, and the two halves run concurrently so the chain is short.
    wd = (3 * width // 8) if width % 8 == 0 else 0

    with tile.TileContext(nc) as tc:
        with tc.tile_pool(name="p", bufs=1) as pool:
            # All input DMAs up front on the SP HWDGE ring (Sync engine does
            # nothing else, so every doorbell rings back-to-back).
            tins = []
            for t in range(n_chunks):
                split = (t in (0, n_chunks - 1)) and width % 2 == 0 \
                    and width >= 1024
                tin = pool.tile([P, width], f32, tag=f"tin{t}")
                src = x[t * P:(t + 1) * P, :]
                if split:
                    nc.sync.dma_start(tin[:, :half], src[:, :half])
                    nc.sync.dma_start(tin[:, half:], src[:, half:])
                else:
                    nc.sync.dma_start(tin[:], src)
                tins.append((tin, split))

            # DVE: rowmax + reciprocal per chunk (partial maxes for split
            # chunks), plus its share of the multiplies.  ACT: only its
            # share of the multiplies.  GpSimd: only the output doorbells
            # (SWDGE) -- an engine with nothing else scheduled on it rings
            # each doorbell the moment the muls' semaphores fire, and the
            # SWDGE descriptor path keeps the output descriptor fetches off
            # the HWDGE rings' descriptor-fetch engine (SDMA 15).
            for t in range(n_chunks):
                tin, split = tins[t]
                m = pool.tile([P, 1], f32, tag=f"m{t}")
                if split:
                    m2 = pool.tile([P, 2], f32, tag=f"m{t}p")
                    nc.vector.reduce_max(m2[:, 0:1], tin[:, :half],
                                         axis=mybir.AxisListType.X)
                    nc.vector.reduce_max(m2[:, 1:2], tin[:, half:],
                                         axis=mybir.AxisListType.X)
                    nc.vector.reduce_max(m[:], m2[:],
                                         axis=mybir.AxisListType.X)
                else:
                    nc.vector.reduce_max(m[:], tin[:],
                                         axis=mybir.AxisListType.X)
                r = pool.tile([P, 1], f32, tag=f"r{t}")
                nc.vector.reciprocal(r[:], m[:])
                if wd:
                    nc.vector.tensor_scalar_mul(tin[:, :wd], tin[:, :wd],
                                                r[:])
                    nc.scalar.mul(tin[:, wd:], tin[:, wd:], r[:])
                else:
                    nc.scalar.mul(tin[:], tin[:], r[:])
                # high_priority pins the doorbell ahead of the next chunk's
                # mul in the scheduler's static order for the in-order ACT
                # engine -- otherwise a later mul whose input DMA is still
                # in flight can head-of-line block this chunk's output.
                with tc.high_priority():
                    nc.scalar.dma_start(y[t * P:(t + 1) * P, :], tin[:])
    nc.compile()
    return nc


def _uniform_width(sample_pos: np.ndarray, n: int):
    """Return segment width W if boundaries are uniform (pos = arange*W)."""
    if sample_pos[0] != 0 or sample_pos[-1] != n:
        return None
    diffs = np.diff(sample_pos)
    if diffs.size == 0 or np.any(diffs != diffs[0]):
        return None
    return int(diffs[0])


def _host_fallback(node_deg: np.ndarray, sample_pos: np.ndarray) -> np.ndarray:
    """Exact mirror of the reference semantics for non-uniform boundaries."""
    import jax

    with jax.default_device(jax.devices("cpu")[0]):
        import jax.numpy as jnp

        deg = jnp.asarray(node_deg)
        pos = jnp.asarray(sample_pos)
        n = deg.shape[0]
        g = pos.shape[0] - 1
        seg_ids = jnp.searchsorted(pos[1:], jnp.arange(n, dtype=pos.dtype),
                                   side="right")
        seg_max = jax.ops.segment_max(deg, seg_ids, num_segments=g)
        return np.asarray(deg / seg_max[seg_ids])


def kernel(node_deg: np.ndarray, sample_pos: np.ndarray) -> np.ndarray:
    global LAST_EXEC_TIME_NS, LAST_RESULTS

    node_deg = np.asarray(node_deg, dtype=np.float32)
    sample_pos = np.asarray(sample_pos, dtype=np.int32)
    n = node_deg.shape[0]
    g = sample_pos.shape[0] - 1

    width = _uniform_width(sample_pos, n)
    if (width is None or width < 512 or g % N_CORES != 0
            or (g // N_CORES) % P != 0):
        return _host_fallback(node_deg, sample_pos)

    segs_per_core = g // N_CORES

    key = (segs_per_core, width)
    if key not in _NC_CACHE:
        _NC_CACHE[key] = _build_uniform_nc(*key)
    nc = _NC_CACHE[key]

    shards = node_deg.reshape(N_CORES, segs_per_core, width)
    in_maps = [{"x": shards[c]} for c in range(N_CORES)]

    trace = bool(int(os.environ.get("KERNEL_TRACE", "0")))
    try:
        res = run_bass_kernel_spmd(nc, in_maps, core_ids=list(range(N_CORES)),
                                   trace=trace)
    except Exception:
        if not trace:
            raise
        # Trace post-processing can fail in sandboxes; results still matter.
        res = run_bass_kernel_spmd(nc, in_maps, core_ids=list(range(N_CORES)),
                                   trace=False)
    LAST_EXEC_TIME_NS = res.exec_time_ns
    LAST_RESULTS = res
    out = np.concatenate([res.results[c]["y"].reshape(-1)
                          for c in range(N_CORES)])
    return out.astype(np.float32, copy=False)
